# revision 9
# baseline (speedup 1.0000x reference)
import sys

sys.path.insert(0, "/opt/trn_rl_repo")

import numpy as np

G, E, N, H = 8, 8192, 512, 32
NP1 = N + 1          # 513
T = N * N            # 262144 tokens per graph
V = H * NP1 * NP1    # flat output elements per graph

L1_MODE = "mix3"     # "mix3" (fp16-hi + bf16-lo 3-pass) or "f32"


# ----------------------------------------------------------------- device code
def build(nc, outs, ins):
    from contextlib import ExitStack

    import concourse.tile as tile
    from concourse import mybir

    f32 = mybir.dt.float32
    f16 = mybir.dt.float16
    bf16 = mybir.dt.bfloat16
    Relu = mybir.ActivationFunctionType.Relu

    out_flat = outs["out"]            # [V] f32, layout (h p q)
    attn3p = ins["attn3p"]            # [3, N*N] f16 (hi/mid/lo12 of inner block)
    strips = ins["strips"]            # [2, NP1] f32 (row0 = attn[0,:], row1 = attn[1:,0] col + pad)
    w2cat2 = ins["w2cat2"]            # [128, 32] f32 (W2 stacked twice)
    ones3 = ins["ones3"]              # [3, 32] f16 (1, 1, 2^-12)
    extw = ins["extw"]                # [2, 32] f32 (ones row; virt row)
    if L1_MODE == "mix3":
        xh = ins["xh"]                # [57, T] f16
        xlb = ins["xlb"]              # [57, T] bf16
        w1h = ins["w1h"]              # [57, 64] f16
        w1lb = ins["w1lb"]            # [57, 64] bf16
    else:
        xf = ins["xf"]                # [57, T] f32
        w1f = ins["w1f"]              # [57, 64] f32

    v3 = out_flat.rearrange("(h p q) -> h p q", h=H, p=NP1, q=NP1)
    v3p = out_flat.rearrange("(h p q) -> p h q", h=H, p=NP1, q=NP1)

    with tile.TileContext(nc) as tc, ExitStack() as ctx:
        cst = ctx.enter_context(tc.tile_pool(name="cst", bufs=1))
        xpool = ctx.enter_context(tc.tile_pool(name="xpool", bufs=3))
        apool = ctx.enter_context(tc.tile_pool(name="apool", bufs=3))
        hpool = ctx.enter_context(tc.tile_pool(name="hpool", bufs=3))
        opool = ctx.enter_context(tc.tile_pool(name="opool", bufs=3))
        ps_l1 = ctx.enter_context(tc.tile_pool(name="ps_l1", bufs=3, space="PSUM"))
        ps_l2 = ctx.enter_context(tc.tile_pool(name="ps_l2", bufs=2, space="PSUM"))

        # ---- constants
        w2_s = cst.tile([128, 32], f32)
        nc.sync.dma_start(out=w2_s[:], in_=w2cat2[:])
        ones3_s = cst.tile([3, 32], f16)
        nc.sync.dma_start(out=ones3_s[:], in_=ones3[:])
        extw_s = cst.tile([2, 32], f32)
        nc.sync.dma_start(out=extw_s[:], in_=extw[:])
        if L1_MODE == "mix3":
            w1h_s = cst.tile([57, 64], f16)
            nc.sync.dma_start(out=w1h_s[:], in_=w1h[:])
            w1lb_s = cst.tile([57, 64], bf16)
            nc.sync.dma_start(out=w1lb_s[:], in_=w1lb[:])
        else:
            w1f_s = cst.tile([57, 64], f32)
            nc.sync.dma_start(out=w1f_s[:], in_=w1f[:])

        # ---- main pass: 16 output rows per block, 4 sub-groups of 4 rows
        RPB = 16                         # rows per block
        CPB = RPB * 512                  # tokens per block
        for blk in range(N // RPB):
            i0 = blk * RPB               # token row base
            c0 = i0 * 512                # token col base into [.., T]

            if L1_MODE == "mix3":
                xh_t = xpool.tile([57, CPB], f16, tag="xh")
                nc.sync.dma_start(out=xh_t[:], in_=xh[:, c0:c0 + CPB])
                xl_t = xpool.tile([57, CPB], bf16, tag="xl")
                nc.sync.dma_start(out=xl_t[:], in_=xlb[:, c0:c0 + CPB])
            else:
                xf_t = xpool.tile([57, CPB], f32, tag="xf")
                nc.sync.dma_start(out=xf_t[:], in_=xf[:, c0:c0 + CPB])
            a3_t = apool.tile([3, CPB], f16, tag="a3")
            nc.gpsimd.dma_start(out=a3_t[:], in_=attn3p[:, c0:c0 + CPB])

            osb = opool.tile([128, RPB // 4 * 512], f32, tag="osb")
            for sub in range(RPB // 4):
                sb = sub * 2048          # token offset of this 4-row group
                hids = []
                for half in range(2):
                    ps1 = ps_l1.tile([128, 512], f32, tag="ps1")
                    s0 = sb + half * 1024
                    s1 = s0 + 512
                    if L1_MODE == "mix3":
                        nc.tensor.matmul(out=ps1[0:64, :], lhsT=w1h_s[:],
                                         rhs=xh_t[:, s0:s0 + 512],
                                         start=True, stop=False, tile_position=(0, 0))
                        nc.tensor.matmul(out=ps1[64:128, :], lhsT=w1h_s[:],
                                         rhs=xh_t[:, s1:s1 + 512],
                                         start=True, stop=False, tile_position=(0, 64))
                        nc.tensor.matmul(out=ps1[0:64, :], lhsT=w1lb_s[:],
                                         rhs=xh_t[:, s0:s0 + 512],
                                         start=False, stop=False, tile_position=(0, 0))
                        nc.tensor.matmul(out=ps1[64:128, :], lhsT=w1lb_s[:],
                                         rhs=xh_t[:, s1:s1 + 512],
                                         start=False, stop=False, tile_position=(0, 64))
                        nc.tensor.matmul(out=ps1[0:64, :], lhsT=w1h_s[:],
                                         rhs=xl_t[:, s0:s0 + 512],
                                         start=False, stop=True, tile_position=(0, 0))
                        nc.tensor.matmul(out=ps1[64:128, :], lhsT=w1h_s[:],
                                         rhs=xl_t[:, s1:s1 + 512],
                                         start=False, stop=True, tile_position=(0, 64))
                    else:
                        nc.tensor.matmul(out=ps1[0:64, :], lhsT=w1f_s[:],
                                         rhs=xf_t[:, s0:s0 + 512],
                                         start=True, stop=True, tile_position=(0, 0))
                        nc.tensor.matmul(out=ps1[64:128, :], lhsT=w1f_s[:],
                                         rhs=xf_t[:, s1:s1 + 512],
                                         start=True, stop=True, tile_position=(0, 64))
                    hid = hpool.tile([128, 512], f32, tag="hid")
                    nc.scalar.activation(out=hid[:], in_=ps1[:], func=Relu)
                    hids.append(hid)

                ps2 = ps_l2.tile([128, 512], f32, tag="ps2")
                for r in range(4):
                    hid = hids[r // 2]
                    base = 64 * (r % 2)
                    nc.tensor.matmul(
                        out=ps2[32 * r:32 * r + 32, :],
                        lhsT=w2_s[base:base + 64, :],
                        rhs=hid[base:base + 64, :],
                        start=True, stop=False,
                        tile_position=(base, 32 * r),
                    )
                for r in range(4):
                    nc.tensor.matmul(
                        out=ps2[32 * r:32 * r + 32, :],
                        lhsT=ones3_s[:],
                        rhs=a3_t[:, sb + r * 512:sb + (r + 1) * 512],
                        start=False, stop=True,
                        tile_position=(0, 32 * r),
                    )
                nc.vector.tensor_copy(
                    out=osb[:, sub * 512:(sub + 1) * 512], in_=ps2[:]
                )
                p0 = i0 + sub * 4
                nc.scalar.dma_start(
                    out=v3p[p0 + 1:p0 + 5, :, 1:513],
                    in_=osb[:, sub * 512:(sub + 1) * 512],
                )

        # ---- column-0 strip (rows 1..512) and row 0
        # col 0, rows 1..512: attn[p,0] + virt
        rhs0 = cst.tile([2, 512], f32)
        nc.gpsimd.memset(rhs0[:], 1.0)
        nc.sync.dma_start(out=rhs0[0:1, :], in_=strips[1:2, 1:513])
        pc0 = ps_l2.tile([32, 512], f32, tag="ps2")
        nc.tensor.matmul(out=pc0[:], lhsT=extw_s[:], rhs=rhs0[:],
                         start=True, stop=True)
        c0sb = opool.tile([32, 512], f32, tag="osb")
        nc.vector.tensor_copy(out=c0sb[:], in_=pc0[:])
        nc.sync.dma_start(out=v3[:, 1:513, 0], in_=c0sb[:])

        # row 0, cols 0..512: attn[0,q] + virt
        rhsr = cst.tile([2, NP1], f32)
        nc.gpsimd.memset(rhsr[:], 1.0)
        nc.sync.dma_start(out=rhsr[0:1, :], in_=strips[0:1, :])
        pr0 = ps_l2.tile([32, 512], f32, tag="ps2")
        nc.tensor.matmul(out=pr0[:], lhsT=extw_s[:], rhs=rhsr[:, 0:512],
                         start=True, stop=True)
        r0sb = opool.tile([32, NP1], f32, tag="osb")
        nc.vector.tensor_copy(out=r0sb[:, 0:512], in_=pr0[:])
        pr1 = ps_l2.tile([32, 1], f32, tag="ps2b")
        nc.tensor.matmul(out=pr1[:], lhsT=extw_s[:], rhs=rhsr[:, 512:513],
                         start=True, stop=True)
        nc.vector.tensor_copy(out=r0sb[:, 512:513], in_=pr1[:])
        nc.sync.dma_start(out=v3[:, 0, :], in_=r0sb[:])


# ----------------------------------------------------------------- host prep
def _split_f16_bf16(a):
    import ml_dtypes
    hi = a.astype(np.float16)
    lo = (a - hi.astype(np.float32)).astype(ml_dtypes.bfloat16)
    return hi, lo


def prep_core(g, inputs):
    attn = np.ascontiguousarray(inputs["attn_bias"][g], np.float32)
    angle = inputs["angle"][g]
    dists = inputs["dists"][g]

    xt = np.empty((57, T), np.float32)
    xt[0:28] = angle.reshape(T, 28).T
    xt[28:56] = dists.reshape(T, 28).T
    xt[56] = 1.0

    w1cat = np.zeros((57, 64), np.float32)
    w1cat[0:28, 0:32] = inputs["ang_w1"]
    w1cat[28:56, 32:64] = inputs["md_w1"]
    w1cat[56, 0:32] = inputs["ang_b1"]
    w1cat[56, 32:64] = inputs["md_b1"]
    w2 = np.concatenate([inputs["ang_w2"], inputs["md_w2"]], 0).astype(np.float32)
    w2cat2 = np.concatenate([w2, w2], 0)
    b2sum = (np.asarray(inputs["ang_b2"]) + np.asarray(inputs["md_b2"])).astype(np.float32)

    # attn inner block, split into fp16 hi/mid/lo*2^12 (exact to ~2^-34)
    a = attn[1:, 1:]                           # [N, N]
    hi = a.astype(np.float16)
    r1 = a - hi.astype(np.float32)
    mid = r1.astype(np.float16)
    r2 = r1 - mid.astype(np.float32)
    lo12 = (r2 * 4096.0).astype(np.float16)
    attn3p = np.stack([hi, mid, lo12]).reshape(3, T)

    strips = np.zeros((2, NP1), np.float32)
    strips[0] = attn[0, :]
    strips[1, 1:] = attn[1:, 0]

    ones3 = np.zeros((3, 32), np.float16)
    ones3[0] = 1.0
    ones3[1] = 1.0
    ones3[2] = 2.0 ** -12

    extw = np.zeros((2, 32), np.float32)
    extw[0] = 1.0
    extw[1] = np.asarray(inputs["virt"], np.float32).reshape(32)

    m = dict(attn3p=attn3p, strips=strips, w2cat2=w2cat2, ones3=ones3,
             extw=extw)
    if L1_MODE == "mix3":
        m["xh"], m["xlb"] = _split_f16_bf16(xt)
        m["w1h"], m["w1lb"] = _split_f16_bf16(w1cat)
    else:
        m["xf"] = xt
        m["w1f"] = w1cat
    return m, b2sum


def edge_emb_host(g, inputs):
    """Edge embeddings + flat scatter indices, computed exactly as reference."""
    ef = np.asarray(inputs["edge_feat"][g])
    ei = np.asarray(inputs["edge_index"][g]).astype(np.int64)
    mask = np.asarray(inputs["edge_mask"][g]).astype(bool)
    nlig = max(int(inputs["num_ligand_atoms"][g]), 1)

    t0 = ef[:, 0].astype(np.int32)
    t1 = ef[:, 1].astype(np.int32)
    t2 = ef[:, 2].astype(np.int32)
    d = ef[:, 3:4].astype(np.float32)          # [E, 1]
    src, tgt = ei[0], ei[1]
    src_l = (src > 0) & (src < nlig)
    tgt_l = (tgt > 0) & (tgt < nlig)

    # distance MLP (f32, same shapes as reference)
    h1 = np.maximum(d @ np.asarray(inputs["dist_w1"], np.float32)
                    + np.asarray(inputs["dist_b1"], np.float32), 0.0)
    demb = h1 @ np.asarray(inputs["dist_w2"], np.float32) \
        + np.asarray(inputs["dist_b2"], np.float32)       # [E, 32]

    sidx = np.clip(t0 * 4 + t1 * 2 + t2, 0, 19)
    structural = np.asarray(inputs["struct_emb"], np.float32)[sidx]
    pidx = np.clip(t1, 0, 14)
    plip = np.where(
        (src_l & tgt_l)[:, None], np.asarray(inputs["plip_lig"], np.float32)[pidx],
        np.where((~src_l & ~tgt_l)[:, None],
                 np.asarray(inputs["plip_prot"], np.float32)[pidx],
                 np.asarray(inputs["plip_inter"], np.float32)[pidx]))
    emb = np.where((t0 <= 1)[:, None], structural,
                   np.where((t0 == 5)[:, None], plip, 0.0)) + demb
    emb = emb * mask[:, None].astype(np.float32)          # [E, 32]

    cell = (src + 1) * NP1 + (tgt + 1)                    # [E]
    h_off = np.arange(H, dtype=np.int64) * (NP1 * NP1)
    idx = cell[:, None] + h_off[None, :]                  # [E, 32]
    return emb, idx


_IN_SPECS_MIX = [
    ("xh", (57, T), "float16"),
    ("xlb", (57, T), "bfloat16"),
    ("w1h", (57, 64), "float16"),
    ("w1lb", (57, 64), "bfloat16"),
]
_IN_SPECS_F32 = [
    ("xf", (57, T), "float32"),
    ("w1f", (57, 64), "float32"),
]
_IN_SPECS_COMMON = [
    ("attn3p", (3, T), "float16"),
    ("strips", (2, NP1), "float32"),
    ("w2cat2", (128, 32), "float32"),
    ("ones3", (3, 32), "float16"),
    ("extw", (2, 32), "float32"),
]


def _build_nc():
    from concourse import bacc, mybir

    nc = bacc.Bacc(
        "TRN2",
        target_bir_lowering=False,
        debug=False,
        enable_asserts=False,
        num_devices=8,
    )
    specs = list(_IN_SPECS_COMMON)
    specs += _IN_SPECS_MIX if L1_MODE == "mix3" else _IN_SPECS_F32
    ins = {}
    for name, shape, dt_name in specs:
        h = nc.dram_tensor(name, list(shape), getattr(mybir.dt, dt_name),
                           kind="ExternalInput")
        ins[name] = h[:]
    out_h = nc.dram_tensor("out", [V], mybir.dt.float32, kind="ExternalOutput")
    build(nc, {"out": out_h[:]}, ins)
    nc.compile()
    return nc


def kernel(_trace=False, **inputs):
    from concourse.bass_utils import run_bass_kernel_spmd

    in_maps = []
    b2sums = []
    edges = []
    for g in range(G):
        m, b2sum = prep_core(g, inputs)
        in_maps.append(m)
        b2sums.append(b2sum)
        edges.append(edge_emb_host(g, inputs))

    nc = _build_nc()
    res = run_bass_kernel_spmd(nc, in_maps, core_ids=list(range(G)), trace=_trace)
    if _trace:
        print("HW exec time:", res.exec_time_ns, "ns  (mean:", res.mean_exec_time_ns,
              "ns, slowest core:", res.max_exec_time_core_id, ")")
        if res.instructions_and_trace:
            print("trace:", res.instructions_and_trace[1])
    outs = []
    for g, r in enumerate(res.results):
        flat = r["out"].copy()
        emb, idx = edges[g]
        np.add.at(flat, idx.ravel(), emb.ravel())
        outs.append(flat.reshape(H, NP1, NP1))
    out = np.stack(outs)
    b2s = np.stack(b2sums)  # [G, 32]
    if np.any(b2s != 0):
        out[:, :, 1:, 1:] += b2s[:, :, None, None]
    return out.astype(np.float32)
